# revision 16
# baseline (speedup 1.0000x reference)
"""Trainium2 Bass kernel for nn_Attn: attn = softmax(enc @ W^T @ hidden^T).

Math: reference computes energy = enc @ W^T + b  ([S,H]), then
attn_energies = energy @ hidden[0]  ([S]), then softmax over S.
Associativity: attn_energies = enc @ (W^T @ hidden^T) + (b . hidden).
The (b . hidden) term is a constant shift over S -> softmax-invariant
(for ANY b), so we drop it.

v3: fp16 everywhere on the wire + TensorEngine matvec + issue-count diet.
  - All model inputs are cast to fp16 ON THE HOST; enc is also
    host-transposed (encT[h, s]).  10.5MB/core -> ~29us HBM stream.
    fp16 rounding puts ~0.02 abs noise on N(0,2048) energies -> lands
    on near-zero softmax weights; measured scale-rel err ~2e-4 vs the
    2e-2 gate.
  - e = enc @ u runs on the otherwise-idle TensorEngine: 64 PSUM-
    accumulated [1,512] matmuls (K=128 h-chunk, M=1, N=512 seq cols).
    u = W^T h is 16 more PE matmuls off the packed wh tile, plus 4
    rank-1 transpose matmuls to turn the [1,512] u row into [128,4]
    PE weight columns.  The DVE only does the tiny combine + softmax.
  - Every dma_start costs ~0.7us of sequencer issue time, so the
    stream is 5 s-major encT blocks [128, 4k, s_blk] (4 descriptors
    per partition) + 2 wh chunks on the scalar ring; each block
    enables all 4 of its k-chunk matmuls at once and the last block
    is only 1MB, so the PE tail past stream-end stays ~2us.
  - Energies are drained to fp16 (ACT/DVE alternating), streamed to
    DRAM per-block on the gpsimd SWDGE ring, and shipped in ONE
    AllGather fired only after the whole enc stream has landed (a
    pending collective stalls in-flight model DMA - prior finding).
    Combine (4 col-partials per row-group) + softmax run redundantly
    on every core; host takes core 0's copy.
"""

import numpy as np

S = 16384
H = 2048
NCORES = 8
RG = 2  # row groups
CG = 4  # column groups
S_LOC = S // RG  # 8192 seq rows per core
H_SH = H // CG  # 512 enc/W columns per core
P = 128
NO = H // P  # 16 contraction chunks for the u matvec
KCH = H_SH // P  # 4 h-chunks per core for the e matvec
WHW = H_SH + 2  # 512 W cols + hidden col + pad (keeps 4B alignment)
SBLKS = [2048, 2048, 2048, 1024, 1024]  # encT s-major DMA blocks

_CACHE = {}


def _build_program():
    import concourse.bacc as bacc
    import concourse.mybir as mybir
    import concourse.tile as tile

    fp32 = mybir.dt.float32
    fp16 = mybir.dt.float16
    nc = bacc.Bacc("TRN2")

    # Host-blocked enc: per s-block bi a contiguous [128, 4, blk] slab
    # with encb[p, k, s] = enc[s0+s, k*128+p] -> ONE 16KB descriptor per
    # partition per block (v3's strided view cost ~11us of descriptor
    # generation on the sequencer per DMA).
    encb_in = nc.dram_tensor("encb", [P, S_LOC * KCH], fp16, kind="ExternalInput")
    # packed per-core weights: wh[p, o, 0:H_SH] = W[o*128+p, c-shard],
    # wh[p, o, H_SH] = hidden[o*128+p], wh[p, o, H_SH+1] = 0 pad.
    wh_in = nc.dram_tensor("wh", [P, NO, WHW], fp16, kind="ExternalInput")
    # fp16 on the wire; the host upcasts to fp32 (weights are in [0,1],
    # adds ~2.4e-4 rel rounding vs the 2e-2 gate)
    attn_out = nc.dram_tensor("attn", [S], fp16, kind="ExternalOutput")

    ident_dram = nc.inline_tensor(np.eye(P, dtype=np.float32), name="ident128")

    groups = [list(range(NCORES))]

    with tile.TileContext(nc) as tc:
        with (
            tc.tile_pool(name="const", bufs=1) as cpool,
            # all 5 encT blocks resident: the whole stream lands before
            # the collective doorbell fires.
            tc.tile_pool(name="encp", bufs=len(SBLKS)) as enc_pool,
            tc.tile_pool(name="small", bufs=1) as small,
            # PSUM budget is 8 banks: eps ring 4 + utps 1 + mix ring 2 = 7.
            tc.tile_pool(name="psA", bufs=1, space="PSUM") as psA,
            tc.tile_pool(name="eps", bufs=4, space="PSUM") as eps,
            tc.tile_pool(name="dram", bufs=1, space="DRAM") as dram,
        ):
            e_part = dram.tile([S_LOC], fp16, name="e_part")
            e_ag = dram.tile([NCORES * S_LOC], fp16, addr_space="Shared", name="e_ag")
            warm_out = dram.tile([NCORES * 64], fp16, addr_space="Shared", name="warm_out")

            # ---- ncfw warmup: a 128B AllGather fired immediately (its
            # input is an inline const, preloaded at NEFF load -> the
            # doorbell rings as soon as the gpsimd queue starts).  The
            # cold collective trigger costs ~11us from last-rank doorbell
            # to ALGO_MESH_BEGIN; a chained collective triggers in ~1us,
            # and the warm mesh also absorbs inter-core start skew before
            # the real AG needs it.  Its pending window spans the whole
            # enc stream; traces show the stream stays at 100% DMA busy
            # (no pending-collective jam).
            warm_in = nc.inline_tensor(
                np.zeros(64, dtype=np.float16), name="warm_seed"
            )
            nc.gpsimd.collective_compute(
                "AllGather",
                mybir.AluOpType.bypass,
                replica_groups=groups,
                ins=[warm_in[:]],
                outs=[warm_out[:]],
            )

            # ---- constants ----
            ident = cpool.tile([P, P], fp32)
            nc.scalar.dma_start(ident[:], ident_dram[:])
            ones_row = cpool.tile([1, P], fp32)  # [K=1, M=128] lhsT: bcast
            nc.vector.memset(ones_row[:], 1.0)
            neg_ones_row = cpool.tile([1, P], fp32)  # bcast with negate
            nc.vector.memset(neg_ones_row[:], -1.0)
            ones_col = cpool.tile([P, 1], fp32)  # [K=128, M=1] lhsT: P-sum
            nc.vector.memset(ones_col[:], 1.0)
            one16 = cpool.tile([1, 1], fp16)  # rhs for the u transposes
            nc.vector.memset(one16[:], 1.0)

            # ---- DMA issue: wh FIRST on the sync ring (FIFO -> u starts
            # early, v2-proven), then the encb blocks. ----
            wh_tiles = []
            for w in range(2):
                wh_t = cpool.tile([P, NO // 2, WHW], fp16, name=f"wh_t{w}")
                nc.sync.dma_start(wh_t[:], wh_in[:, w * (NO // 2) : (w + 1) * (NO // 2), :])
                wh_tiles.append(wh_t)
            enc_tiles = []
            s0 = 0
            for bi, blk in enumerate(SBLKS):
                enc_t = enc_pool.tile([P, KCH, blk], fp16, tag="encT", name=f"enc{bi}")
                src = encb_in[:, s0 * KCH : (s0 + blk) * KCH]
                nc.sync.dma_start(enc_t[:], src.rearrange("p (k s) -> p k s", k=KCH))
                enc_tiles.append((enc_t, s0))
                s0 += blk

            # ---- u = hidden @ W[:, c-shard] on the PE ----
            # 16 chained matmuls (K=128 hidden chunk, M=1, N=512) into one
            # PSUM bank; the first 8 fire as wh chunk 0 lands.
            u_ps = eps.tile([1, H_SH], fp32, tag="eps", name="u_ps")
            for o in range(NO):
                wh_t = wh_tiles[o // (NO // 2)]
                oo = o % (NO // 2)
                nc.tensor.matmul(
                    u_ps[:],
                    wh_t[:, oo, H_SH : H_SH + 1],
                    wh_t[:, oo, 0:H_SH],
                    start=(o == 0),
                    stop=(o == NO - 1),
                )
            u_row = small.tile([1, H_SH], fp16)
            nc.scalar.copy(u_row[:], u_ps[:])
            # [1,512] row -> [128,4] columns: 4 rank-1 transpose matmuls
            # (lhsT = u_row slice [K=1, M=128], rhs = [1,1] one).
            uT_ps = psA.tile([P, KCH], fp32, tag="utps")
            for m in range(KCH):
                nc.tensor.matmul(
                    uT_ps[:, m : m + 1],
                    u_row[:, m * P : (m + 1) * P],
                    one16[:],
                    start=True,
                    stop=True,
                )
            uT = small.tile([P, KCH], fp16)
            nc.scalar.copy(uT[:], uT_ps[:])

            # ---- e = encT.T @ u on the PE, block by block ----
            # k-outer per block: all 4 k-matmuls of a block enable the
            # moment the block lands.  Drains alternate ACT/DVE (fp16 out);
            # each block's energies stream to DRAM on the gpsimd SWDGE
            # ring so the final store before the doorbell is tiny.
            e_sb = small.tile([1, S_LOC], fp16)
            for bi, (enc_t, s0) in enumerate(enc_tiles):
                blk = SBLKS[bi]
                nsl = blk // 512
                es = [
                    eps.tile([1, 512], fp32, tag="eps", name=f"eps{bi}{i}")
                    for i in range(nsl)
                ]
                for k in range(KCH):
                    for i in range(nsl):
                        nc.tensor.matmul(
                            es[i][:],
                            uT[:, k : k + 1],
                            enc_t[:, k, i * 512 : (i + 1) * 512],
                            start=(k == 0),
                            stop=(k == KCH - 1),
                        )
                for i in range(nsl):
                    dst = e_sb[:, s0 + i * 512 : s0 + (i + 1) * 512]
                    if i % 2 == 0:
                        nc.scalar.copy(dst, es[i][:])
                    else:
                        nc.vector.tensor_copy(dst, es[i][:])
                nc.scalar.dma_start(
                    e_part[s0 : s0 + blk].rearrange("(a c) -> a c", a=1),
                    e_sb[:, s0 : s0 + blk],
                )

            # ---- ship energies: ONE AllGather, fired post-stream ----
            nc.gpsimd.collective_compute(
                "AllGather",
                mybir.AluOpType.bypass,
                replica_groups=groups,
                ins=[e_part[:]],
                outs=[e_ag[:]],
            )

            # ---- combine column partials, then softmax (redundant) ----
            # e_ag = (r i): rank r = g*4+c holds partial e for s = g*8192+i,
            # i = p*64 + c''.  ea[p, j]: j = g*64 + c'' -> s = g*8192+p*64+c''.
            # Two half-loads on different rings drain in parallel.
            e_ag_v = e_ag[:].rearrange("(r p c) -> p r c", r=NCORES, p=P)
            parts = small.tile([P, NCORES, 64], fp16)
            nc.scalar.dma_start(parts[:, 0 : NCORES // 2, :], e_ag_v[:, 0 : NCORES // 2])
            nc.sync.dma_start(parts[:, NCORES // 2 :, :], e_ag_v[:, NCORES // 2 :])
            qq = small.tile([P, NCORES // 2, 64], fp32)
            parts_v = parts[:].rearrange("p (r2 x) c -> p r2 x c", x=2)
            nc.vector.tensor_add(qq[:], parts_v[:, :, 0, :], parts_v[:, :, 1, :])
            ea = small.tile([P, S // P], fp32)
            qq_v = qq[:].rearrange("p (g x) c -> p g x c", x=2)
            ea_v = ea[:].rearrange("p (g c) -> p g c", g=RG)
            nc.vector.tensor_add(ea_v[:], qq_v[:, :, 0, :], qq_v[:, :, 1, :])

            mx = small.tile([P, 1], fp32)
            nc.vector.reduce_max(mx[:], ea[:], axis=mybir.AxisListType.X)
            # global max: transpose [128,1] -> [1,128] on PE, reduce row 0
            mrow_ps = psA.tile([1, P], fp32, tag="mix", name="mrow_ps")
            nc.tensor.transpose(mrow_ps[:], mx[:], ident[:])
            gmax = small.tile([1, 1], fp32)
            nc.vector.reduce_max(gmax[:], mrow_ps[:], axis=mybir.AxisListType.X)
            # broadcast -gmax to [128,1] (negated ones fold the sign)
            gb_ps = psA.tile([P, 1], fp32, tag="mix", name="gb_ps")
            nc.tensor.matmul(gb_ps[:], neg_ones_row[:], gmax[:])
            nmx = small.tile([P, 1], fp32)
            nc.scalar.copy(nmx[:], gb_ps[:])
            # exp(e - gmax) with per-partition row sums in one ACT op
            xs = small.tile([P, S // P], fp32)
            sums = small.tile([P, 1], fp32)
            nc.scalar.activation(
                xs[:],
                ea[:],
                mybir.ActivationFunctionType.Exp,
                bias=nmx[:],
                scale=1.0,
                accum_out=sums[:],
            )
            # global sum: contract the partition axis on the PE
            tot_ps = psA.tile([1, 1], fp32, tag="mix", name="tot_ps")
            nc.tensor.matmul(tot_ps[:], ones_col[:], sums[:])
            rec = small.tile([1, 1], fp32)
            nc.vector.reciprocal(rec[:], tot_ps[:])
            rb_ps = psA.tile([P, 1], fp32, tag="mix", name="rb_ps")
            nc.tensor.matmul(rb_ps[:], ones_row[:], rec[:])
            outx = small.tile([P, S // P], fp16)
            nc.vector.tensor_scalar_mul(outx[:], xs[:], rb_ps[:])
            # j = g*64 + c'' -> s = g*8192 + p*64 + c''; one row-group
            # half per ring so the two stores drain in parallel
            out_v = attn_out.rearrange("(g p c) -> g p c", g=RG, p=P)
            outx_v = outx[:].rearrange("p (g c) -> g p c", g=RG)
            nc.sync.dma_start(out_v[0], outx_v[0])
            nc.scalar.dma_start(out_v[1], outx_v[1])

    nc.compile()
    return nc


def _get_program():
    if "nc" not in _CACHE:
        _CACHE["nc"] = _build_program()
    return _CACHE["nc"]


def _make_in_maps(hidden, encoder_outputs, W):
    hidden = np.asarray(hidden, dtype=np.float32).astype(np.float16)
    enc = np.asarray(encoder_outputs, dtype=np.float32).astype(np.float16)
    W = np.asarray(W, dtype=np.float32).astype(np.float16)
    hid = hidden.reshape(NO, P).transpose(1, 0)  # hid[p, o] = hidden[o*128+p]
    # W as [p, o, h]: W_poh[p, o, h] = W[o*128+p, h]
    W_poh = W.reshape(NO, P, H).transpose(1, 0, 2)
    in_maps = []
    for r in range(NCORES):
        g, c = divmod(r, CG)
        wh = np.zeros((P, NO, WHW), dtype=np.float16)
        wh[:, :, 0:H_SH] = W_poh[:, :, c * H_SH : (c + 1) * H_SH]
        wh[:, :, H_SH] = hid
        shard = enc[g * S_LOC : (g + 1) * S_LOC, c * H_SH : (c + 1) * H_SH]
        # block-major slabs: encb[p, (blocks: k, s)] = enc[s0+s, k*128+p]
        encb = np.empty((P, S_LOC * KCH), dtype=np.float16)
        s0 = 0
        for blk in SBLKS:
            seg = shard[s0 : s0 + blk, :].T.reshape(KCH, P, blk).transpose(1, 0, 2)
            encb[:, s0 * KCH : (s0 + blk) * KCH] = seg.reshape(P, KCH * blk)
            s0 += blk
        in_maps.append({"encb": encb, "wh": wh})
    return in_maps


def run(hidden, encoder_outputs, W, b=None, trace=False):
    from concourse.bass_utils import run_bass_kernel_spmd

    nc = _get_program()
    in_maps = _make_in_maps(hidden, encoder_outputs, W)
    res = run_bass_kernel_spmd(nc, in_maps, list(range(NCORES)), trace=trace)
    out = np.asarray(res.results[0]["attn"], dtype=np.float32).reshape(1, 1, S)
    return out, res


def kernel(hidden, encoder_outputs, W, b):
    out, _ = run(hidden, encoder_outputs, W, b)
    return out


# revision 18
# speedup vs baseline: 1.1847x; 1.1847x over previous
"""Trainium2 Bass kernel for nn_Attn: attn = softmax(enc @ W^T @ hidden^T).

Math: reference computes energy = enc @ W^T + b  ([S,H]), then
attn_energies = energy @ hidden[0]  ([S]), then softmax over S.
Associativity: attn_energies = enc @ (W^T @ hidden^T) + (b . hidden).
The (b . hidden) term is a constant shift over S -> softmax-invariant
(for ANY b), so we drop it.

v3: fp16 everywhere on the wire + TensorEngine matvec + issue-count diet.
  - All model inputs are cast to fp16 ON THE HOST; enc is also
    host-transposed (encT[h, s]).  10.5MB/core -> ~29us HBM stream.
    fp16 rounding puts ~0.02 abs noise on N(0,2048) energies -> lands
    on near-zero softmax weights; measured scale-rel err ~2e-4 vs the
    2e-2 gate.
  - e = enc @ u runs on the otherwise-idle TensorEngine: 64 PSUM-
    accumulated [1,512] matmuls (K=128 h-chunk, M=1, N=512 seq cols).
    u = W^T h is 16 more PE matmuls off the packed wh tile, plus 4
    rank-1 transpose matmuls to turn the [1,512] u row into [128,4]
    PE weight columns.  The DVE only does the tiny combine + softmax.
  - Every dma_start costs ~0.7us of sequencer issue time, so the
    stream is 5 s-major encT blocks [128, 4k, s_blk] (4 descriptors
    per partition) + 2 wh chunks on the scalar ring; each block
    enables all 4 of its k-chunk matmuls at once and the last block
    is only 1MB, so the PE tail past stream-end stays ~2us.
  - Energies are drained to fp16 (ACT/DVE alternating), streamed to
    DRAM per-block on the gpsimd SWDGE ring, and shipped in ONE
    AllGather fired only after the whole enc stream has landed (a
    pending collective stalls in-flight model DMA - prior finding).
    Combine (4 col-partials per row-group) + softmax run redundantly
    on every core; host takes core 0's copy.
"""

import numpy as np

S = 16384
H = 2048
NCORES = 8
RG = 2  # row groups
CG = 4  # column groups
S_LOC = S // RG  # 8192 seq rows per core
H_SH = H // CG  # 512 enc/W columns per core
P = 128
NO = H // P  # 16 contraction chunks for the u matvec
KCH = H_SH // P  # 4 h-chunks per core for the e matvec
WHW = H_SH + 2  # 512 W cols + hidden col + pad (keeps 4B alignment)
SBLKS = [2048, 2048, 2048, 1024, 1024]  # encT s-major DMA blocks

_CACHE = {}


def _build_program():
    import concourse.bacc as bacc
    import concourse.mybir as mybir
    import concourse.tile as tile

    fp32 = mybir.dt.float32
    fp16 = mybir.dt.float16
    nc = bacc.Bacc("TRN2")

    # Host-blocked enc: per s-block bi a contiguous [128, 4, blk] slab
    # with encb[p, k, s] = enc[s0+s, k*128+p] -> ONE 16KB descriptor per
    # partition per block (v3's strided view cost ~11us of descriptor
    # generation on the sequencer per DMA).
    encb_in = nc.dram_tensor("encb", [P, S_LOC * KCH], fp16, kind="ExternalInput")
    # packed per-core weights: wh[p, o, 0:H_SH] = W[o*128+p, c-shard],
    # wh[p, o, H_SH] = hidden[o*128+p], wh[p, o, H_SH+1] = 0 pad.
    wh_in = nc.dram_tensor("wh", [P, NO, WHW], fp16, kind="ExternalInput")
    # fp16 on the wire; the host upcasts to fp32 (weights are in [0,1],
    # adds ~2.4e-4 rel rounding vs the 2e-2 gate)
    attn_out = nc.dram_tensor("attn", [S], fp16, kind="ExternalOutput")

    ident_dram = nc.inline_tensor(np.eye(P, dtype=np.float32), name="ident128")

    groups = [list(range(NCORES))]

    with tile.TileContext(nc) as tc:
        with (
            tc.tile_pool(name="const", bufs=1) as cpool,
            # all 5 encT blocks resident: the whole stream lands before
            # the collective doorbell fires.
            tc.tile_pool(name="encp", bufs=len(SBLKS)) as enc_pool,
            tc.tile_pool(name="small", bufs=1) as small,
            # PSUM budget is 8 banks: eps ring 4 + utps 1 + mix ring 2 = 7.
            tc.tile_pool(name="psA", bufs=1, space="PSUM") as psA,
            tc.tile_pool(name="eps", bufs=4, space="PSUM") as eps,
            tc.tile_pool(name="dram", bufs=1, space="DRAM") as dram,
        ):
            e_part = dram.tile([S_LOC], fp16, name="e_part")
            e_ag = dram.tile([NCORES * S_LOC], fp16, addr_space="Shared", name="e_ag")
            warm_out = dram.tile([NCORES * 64], fp16, addr_space="Shared", name="warm_out")

            # ---- ncfw warmup: a 128B AllGather fired immediately (its
            # input is an inline const, preloaded at NEFF load -> the
            # doorbell rings as soon as the gpsimd queue starts).  The
            # cold collective trigger costs ~11us from last-rank doorbell
            # to ALGO_MESH_BEGIN; a chained collective triggers in ~1us,
            # and the warm mesh also absorbs inter-core start skew before
            # the real AG needs it.  Its pending window spans the whole
            # enc stream; traces show the stream stays at 100% DMA busy
            # (no pending-collective jam).
            warm_in = nc.inline_tensor(
                np.zeros(64, dtype=np.float16), name="warm_seed"
            )
            nc.gpsimd.collective_compute(
                "AllGather",
                mybir.AluOpType.bypass,
                replica_groups=groups,
                ins=[warm_in[:]],
                outs=[warm_out[:]],
            )

            # ---- constants ----
            ident = cpool.tile([P, P], fp32)
            nc.scalar.dma_start(ident[:], ident_dram[:])
            ones_row = cpool.tile([1, P], fp32)  # [K=1, M=128] lhsT: bcast
            nc.vector.memset(ones_row[:], 1.0)
            neg_ones_row = cpool.tile([1, P], fp32)  # bcast with negate
            nc.vector.memset(neg_ones_row[:], -1.0)
            ones_col = cpool.tile([P, 1], fp32)  # [K=128, M=1] lhsT: P-sum
            nc.vector.memset(ones_col[:], 1.0)
            one16 = cpool.tile([1, 1], fp16)  # rhs for the u transposes
            nc.vector.memset(one16[:], 1.0)

            # ---- DMA issue: wh FIRST on the sync ring (FIFO -> u starts
            # early, v2-proven), then the encb blocks. ----
            wh_tiles = []
            for w in range(2):
                wh_t = cpool.tile([P, NO // 2, WHW], fp16, name=f"wh_t{w}")
                nc.sync.dma_start(wh_t[:], wh_in[:, w * (NO // 2) : (w + 1) * (NO // 2), :])
                wh_tiles.append(wh_t)
            enc_tiles = []
            s0 = 0
            for bi, blk in enumerate(SBLKS):
                enc_t = enc_pool.tile([P, KCH, blk], fp16, tag="encT", name=f"enc{bi}")
                src = encb_in[:, s0 * KCH : (s0 + blk) * KCH]
                nc.sync.dma_start(enc_t[:], src.rearrange("p (k s) -> p k s", k=KCH))
                enc_tiles.append((enc_t, s0))
                s0 += blk

            # ---- u = hidden @ W[:, c-shard] on the PE ----
            # 16 chained matmuls (K=128 hidden chunk, M=1, N=512) into one
            # PSUM bank; the first 8 fire as wh chunk 0 lands.
            u_ps = eps.tile([1, H_SH], fp32, tag="eps", name="u_ps")
            for o in range(NO):
                wh_t = wh_tiles[o // (NO // 2)]
                oo = o % (NO // 2)
                nc.tensor.matmul(
                    u_ps[:],
                    wh_t[:, oo, H_SH : H_SH + 1],
                    wh_t[:, oo, 0:H_SH],
                    start=(o == 0),
                    stop=(o == NO - 1),
                )
            u_row = small.tile([1, H_SH], fp16)
            nc.scalar.copy(u_row[:], u_ps[:])
            # [1,512] row -> [128,4] columns: 4 rank-1 transpose matmuls
            # (lhsT = u_row slice [K=1, M=128], rhs = [1,1] one).
            uT_ps = psA.tile([P, KCH], fp32, tag="utps")
            for m in range(KCH):
                nc.tensor.matmul(
                    uT_ps[:, m : m + 1],
                    u_row[:, m * P : (m + 1) * P],
                    one16[:],
                    start=True,
                    stop=True,
                )
            uT = small.tile([P, KCH], fp16)
            nc.scalar.copy(uT[:], uT_ps[:])

            # ---- e = encT.T @ u on the PE, block by block ----
            # k-outer per block: all 4 k-matmuls of a block enable the
            # moment the block lands.  Drains alternate ACT/DVE (fp16 out);
            # each block's energies stream to DRAM on the gpsimd SWDGE
            # ring so the final store before the doorbell is tiny.
            e_sb = small.tile([1, S_LOC], fp16)
            for bi, (enc_t, s0) in enumerate(enc_tiles):
                blk = SBLKS[bi]
                nsl = blk // 512
                es = [
                    eps.tile([1, 512], fp32, tag="eps", name=f"eps{bi}{i}")
                    for i in range(nsl)
                ]
                for k in range(KCH):
                    for i in range(nsl):
                        nc.tensor.matmul(
                            es[i][:],
                            uT[:, k : k + 1],
                            enc_t[:, k, i * 512 : (i + 1) * 512],
                            start=(k == 0),
                            stop=(k == KCH - 1),
                        )
                for i in range(nsl):
                    dst = e_sb[:, s0 + i * 512 : s0 + (i + 1) * 512]
                    if i % 2 == 0:
                        nc.scalar.copy(dst, es[i][:])
                    else:
                        nc.vector.tensor_copy(dst, es[i][:])
                nc.scalar.dma_start(
                    e_part[s0 : s0 + blk].rearrange("(a c) -> a c", a=1),
                    e_sb[:, s0 : s0 + blk],
                )

            # ---- ship energies: ONE AllGather, fired post-stream ----
            nc.gpsimd.collective_compute(
                "AllGather",
                mybir.AluOpType.bypass,
                replica_groups=groups,
                ins=[e_part[:]],
                outs=[e_ag[:]],
            )

            # ---- combine column partials, then softmax (redundant) ----
            # e_ag = (r i): rank r = g*4+c holds partial e for s = g*8192+i,
            # i = p*64 + c''.  ea[p, j]: j = g*64 + c'' -> s = g*8192+p*64+c''.
            # Two half-loads on different rings drain in parallel.
            e_ag_v = e_ag[:].rearrange("(r p c) -> p r c", r=NCORES, p=P)
            parts = small.tile([P, NCORES, 64], fp16)
            nc.scalar.dma_start(parts[:, 0 : NCORES // 2, :], e_ag_v[:, 0 : NCORES // 2])
            nc.sync.dma_start(parts[:, NCORES // 2 :, :], e_ag_v[:, NCORES // 2 :])
            qq = small.tile([P, NCORES // 2, 64], fp32)
            parts_v = parts[:].rearrange("p (r2 x) c -> p r2 x c", x=2)
            nc.vector.tensor_add(qq[:], parts_v[:, :, 0, :], parts_v[:, :, 1, :])
            ea = small.tile([P, S // P], fp32)
            qq_v = qq[:].rearrange("p (g x) c -> p g x c", x=2)
            ea_v = ea[:].rearrange("p (g c) -> p g c", g=RG)
            nc.vector.tensor_add(ea_v[:], qq_v[:, :, 0, :], qq_v[:, :, 1, :])

            mx = small.tile([P, 1], fp32)
            nc.vector.reduce_max(mx[:], ea[:], axis=mybir.AxisListType.X)
            # global max: transpose [128,1] -> [1,128] on PE, reduce row 0
            mrow_ps = psA.tile([1, P], fp32, tag="mix", name="mrow_ps")
            nc.tensor.transpose(mrow_ps[:], mx[:], ident[:])
            gmax = small.tile([1, 1], fp32)
            nc.vector.reduce_max(gmax[:], mrow_ps[:], axis=mybir.AxisListType.X)
            # broadcast -gmax to [128,1] (negated ones fold the sign)
            gb_ps = psA.tile([P, 1], fp32, tag="mix", name="gb_ps")
            nc.tensor.matmul(gb_ps[:], neg_ones_row[:], gmax[:])
            nmx = small.tile([P, 1], fp32)
            nc.scalar.copy(nmx[:], gb_ps[:])
            # exp(e - gmax) with per-partition row sums in one ACT op
            xs = small.tile([P, S // P], fp32)
            sums = small.tile([P, 1], fp32)
            nc.scalar.activation(
                xs[:],
                ea[:],
                mybir.ActivationFunctionType.Exp,
                bias=nmx[:],
                scale=1.0,
                accum_out=sums[:],
            )
            # global sum: contract the partition axis on the PE
            tot_ps = psA.tile([1, 1], fp32, tag="mix", name="tot_ps")
            nc.tensor.matmul(tot_ps[:], ones_col[:], sums[:])
            rec = small.tile([1, 1], fp32)
            nc.vector.reciprocal(rec[:], tot_ps[:])
            rb_ps = psA.tile([P, 1], fp32, tag="mix", name="rb_ps")
            nc.tensor.matmul(rb_ps[:], ones_row[:], rec[:])
            outx = small.tile([P, S // P], fp16)
            nc.vector.tensor_scalar_mul(outx[:], xs[:], rb_ps[:])
            # j = g*64 + c'' -> s = g*8192 + p*64 + c''; one row-group
            # half per ring so the two stores drain in parallel
            out_v = attn_out.rearrange("(g p c) -> g p c", g=RG, p=P)
            outx_v = outx[:].rearrange("p (g c) -> g p c", g=RG)
            nc.sync.dma_start(out_v[0], outx_v[0])
            nc.scalar.dma_start(out_v[1], outx_v[1])

    nc.compile()
    return nc


def _get_program():
    if "nc" not in _CACHE:
        _CACHE["nc"] = _build_program()
    return _CACHE["nc"]


def _make_in_maps(hidden, encoder_outputs, W):
    hidden = np.asarray(hidden, dtype=np.float32).astype(np.float16)
    enc = np.asarray(encoder_outputs, dtype=np.float32).astype(np.float16)
    W = np.asarray(W, dtype=np.float32).astype(np.float16)
    hid = hidden.reshape(NO, P).transpose(1, 0)  # hid[p, o] = hidden[o*128+p]
    # W as [p, o, h]: W_poh[p, o, h] = W[o*128+p, h]
    W_poh = W.reshape(NO, P, H).transpose(1, 0, 2)
    in_maps = []
    for r in range(NCORES):
        g, c = divmod(r, CG)
        wh = np.zeros((P, NO, WHW), dtype=np.float16)
        wh[:, :, 0:H_SH] = W_poh[:, :, c * H_SH : (c + 1) * H_SH]
        wh[:, :, H_SH] = hid
        shard = enc[g * S_LOC : (g + 1) * S_LOC, c * H_SH : (c + 1) * H_SH]
        # block-major slabs: encb[p, (blocks: k, s)] = enc[s0+s, k*128+p]
        encb = np.empty((P, S_LOC * KCH), dtype=np.float16)
        s0 = 0
        for blk in SBLKS:
            seg = shard[s0 : s0 + blk, :].T.reshape(KCH, P, blk).transpose(1, 0, 2)
            encb[:, s0 * KCH : (s0 + blk) * KCH] = seg.reshape(P, KCH * blk)
            s0 += blk
        in_maps.append({"encb": encb, "wh": wh})
    return in_maps


def run(hidden, encoder_outputs, W, b=None, trace=False):
    from concourse.bass_utils import run_bass_kernel_spmd

    nc = _get_program()
    in_maps = _make_in_maps(hidden, encoder_outputs, W)
    res = run_bass_kernel_spmd(nc, in_maps, list(range(NCORES)), trace=trace)
    out = np.asarray(res.results[0]["attn"], dtype=np.float32).reshape(1, 1, S)
    return out, res


def kernel(hidden, encoder_outputs, W, b):
    out, _ = run(hidden, encoder_outputs, W, b)
    return out


# revision 20
# speedup vs baseline: 1.2867x; 1.0861x over previous
"""Trainium2 Bass kernel for nn_Attn: attn = softmax(enc @ W^T @ hidden^T).

Math: reference computes energy = enc @ W^T + b  ([S,H]), then
attn_energies = energy @ hidden[0]  ([S]), then softmax over S.
Associativity: attn_energies = enc @ (W^T @ hidden^T) + (b . hidden).
The (b . hidden) term is a constant shift over S -> softmax-invariant
(for ANY b), so we drop it.

v3: fp16 everywhere on the wire + TensorEngine matvec + issue-count diet.
  - All model inputs are cast to fp16 ON THE HOST; enc is also
    host-transposed (encT[h, s]).  10.5MB/core -> ~29us HBM stream.
    fp16 rounding puts ~0.02 abs noise on N(0,2048) energies -> lands
    on near-zero softmax weights; measured scale-rel err ~2e-4 vs the
    2e-2 gate.
  - e = enc @ u runs on the otherwise-idle TensorEngine: 64 PSUM-
    accumulated [1,512] matmuls (K=128 h-chunk, M=1, N=512 seq cols).
    u = W^T h is 16 more PE matmuls off the packed wh tile, plus 4
    rank-1 transpose matmuls to turn the [1,512] u row into [128,4]
    PE weight columns.  The DVE only does the tiny combine + softmax.
  - Every dma_start costs ~0.7us of sequencer issue time, so the
    stream is 5 s-major encT blocks [128, 4k, s_blk] (4 descriptors
    per partition) + 2 wh chunks on the scalar ring; each block
    enables all 4 of its k-chunk matmuls at once and the last block
    is only 1MB, so the PE tail past stream-end stays ~2us.
  - Energies are drained to fp16 (ACT/DVE alternating), streamed to
    DRAM per-block on the gpsimd SWDGE ring, and shipped in ONE
    AllGather fired only after the whole enc stream has landed (a
    pending collective stalls in-flight model DMA - prior finding).
    Combine (4 col-partials per row-group) + softmax run redundantly
    on every core; host takes core 0's copy.
"""

import numpy as np

S = 16384
H = 2048
NCORES = 8
RG = 2  # row groups
CG = 4  # column groups
S_LOC = S // RG  # 8192 seq rows per core
H_SH = H // CG  # 512 enc/W columns per core
P = 128
NO = H // P  # 16 contraction chunks for the u matvec
KCH = H_SH // P  # 4 h-chunks per core for the e matvec
WHW = H_SH + 2  # 512 W cols + hidden col + pad (keeps 4B alignment)
SBLKS = [2048, 2048, 2048, 1024, 1024]  # encT s-major DMA blocks

_CACHE = {}


def _build_program():
    import concourse.bacc as bacc
    import concourse.mybir as mybir
    import concourse.tile as tile

    fp32 = mybir.dt.float32
    fp16 = mybir.dt.float16
    nc = bacc.Bacc("TRN2")

    # Host-blocked enc: per s-block bi a contiguous [128, 4, blk] slab
    # with encb[p, k, s] = enc[s0+s, k*128+p] -> ONE 16KB descriptor per
    # partition per block (v3's strided view cost ~11us of descriptor
    # generation on the sequencer per DMA).
    encb_in = nc.dram_tensor("encb", [P, S_LOC * KCH], fp16, kind="ExternalInput")
    # packed per-core weights: wh[p, o, 0:H_SH] = W[o*128+p, c-shard],
    # wh[p, o, H_SH] = hidden[o*128+p], wh[p, o, H_SH+1] = 0 pad.
    wh_in = nc.dram_tensor("wh", [P, NO, WHW], fp16, kind="ExternalInput")
    # fp16 on the wire; the host upcasts to fp32 (weights are in [0,1],
    # adds ~2.4e-4 rel rounding vs the 2e-2 gate)
    attn_out = nc.dram_tensor("attn", [S], fp16, kind="ExternalOutput")

    ident_dram = nc.inline_tensor(np.eye(P, dtype=np.float32), name="ident128")

    groups = [list(range(NCORES))]

    with tile.TileContext(nc) as tc:
        with (
            tc.tile_pool(name="const", bufs=1) as cpool,
            # all 5 encT blocks resident: the whole stream lands before
            # the collective doorbell fires.
            tc.tile_pool(name="encp", bufs=len(SBLKS)) as enc_pool,
            tc.tile_pool(name="small", bufs=1) as small,
            # PSUM budget is 8 banks: eps ring 4 + utps 1 + mix ring 2 = 7.
            tc.tile_pool(name="psA", bufs=1, space="PSUM") as psA,
            tc.tile_pool(name="eps", bufs=4, space="PSUM") as eps,
            tc.tile_pool(name="dram", bufs=1, space="DRAM") as dram,
        ):
            e_part = dram.tile([S_LOC], fp16, name="e_part")
            e_ag = dram.tile([NCORES * S_LOC], fp16, addr_space="Shared", name="e_ag")
            warm_out = dram.tile([NCORES * 64], fp16, addr_space="Shared", name="warm_out")

            # ---- ncfw warmup: a 128B AllGather fired immediately (its
            # input is an inline const, preloaded at NEFF load -> the
            # doorbell rings as soon as the gpsimd queue starts).  The
            # cold collective trigger costs ~11us from last-rank doorbell
            # to ALGO_MESH_BEGIN; a chained collective triggers in ~1us,
            # and the warm mesh also absorbs inter-core start skew before
            # the real AG needs it.  Its pending window spans the whole
            # enc stream; traces show the stream stays at 100% DMA busy
            # (no pending-collective jam).
            warm_in = nc.inline_tensor(
                np.zeros(64, dtype=np.float16), name="warm_seed"
            )
            nc.gpsimd.collective_compute(
                "AllGather",
                mybir.AluOpType.bypass,
                replica_groups=groups,
                ins=[warm_in[:]],
                outs=[warm_out[:]],
            )

            # ---- constants ----
            ident = cpool.tile([P, P], fp32)
            nc.scalar.dma_start(ident[:], ident_dram[:])
            ones_row = cpool.tile([1, P], fp32)  # [K=1, M=128] lhsT: bcast
            nc.vector.memset(ones_row[:], 1.0)
            neg_ones_row = cpool.tile([1, P], fp32)  # bcast with negate
            nc.vector.memset(neg_ones_row[:], -1.0)
            ones_col = cpool.tile([P, 1], fp32)  # [K=128, M=1] lhsT: P-sum
            nc.vector.memset(ones_col[:], 1.0)
            one16 = cpool.tile([1, 1], fp16)  # rhs for the u transposes
            nc.vector.memset(one16[:], 1.0)

            # ---- DMA issue: wh FIRST on the sync ring (FIFO -> u starts
            # early, v2-proven), then the encb blocks. ----
            wh_tiles = []
            for w in range(2):
                wh_t = cpool.tile([P, NO // 2, WHW], fp16, name=f"wh_t{w}")
                nc.sync.dma_start(wh_t[:], wh_in[:, w * (NO // 2) : (w + 1) * (NO // 2), :])
                wh_tiles.append(wh_t)
            enc_tiles = []
            s0 = 0
            for bi, blk in enumerate(SBLKS):
                enc_t = enc_pool.tile([P, KCH, blk], fp16, tag="encT", name=f"enc{bi}")
                src = encb_in[:, s0 * KCH : (s0 + blk) * KCH]
                nc.sync.dma_start(enc_t[:], src.rearrange("p (k s) -> p k s", k=KCH))
                enc_tiles.append((enc_t, s0))
                s0 += blk

            # ---- u = hidden @ W[:, c-shard] on the PE ----
            # 16 chained matmuls (K=128 hidden chunk, M=1, N=512) into one
            # PSUM bank; the first 8 fire as wh chunk 0 lands.
            u_ps = eps.tile([1, H_SH], fp32, tag="eps", name="u_ps")
            for o in range(NO):
                wh_t = wh_tiles[o // (NO // 2)]
                oo = o % (NO // 2)
                nc.tensor.matmul(
                    u_ps[:],
                    wh_t[:, oo, H_SH : H_SH + 1],
                    wh_t[:, oo, 0:H_SH],
                    start=(o == 0),
                    stop=(o == NO - 1),
                )
            u_row = small.tile([1, H_SH], fp16)
            nc.scalar.copy(u_row[:], u_ps[:])
            # [1,512] row -> [128,4] columns: 4 rank-1 transpose matmuls
            # (lhsT = u_row slice [K=1, M=128], rhs = [1,1] one).
            uT_ps = psA.tile([P, KCH], fp32, tag="utps")
            for m in range(KCH):
                nc.tensor.matmul(
                    uT_ps[:, m : m + 1],
                    u_row[:, m * P : (m + 1) * P],
                    one16[:],
                    start=True,
                    stop=True,
                )
            uT = small.tile([P, KCH], fp16)
            nc.scalar.copy(uT[:], uT_ps[:])

            # ---- e = encT.T @ u on the PE, block by block ----
            # k-outer per block: all 4 k-matmuls of a block enable the
            # moment the block lands.  Drains alternate ACT/DVE (fp16 out);
            # each block's energies stream to DRAM on the gpsimd SWDGE
            # ring so the final store before the doorbell is tiny.
            e_sb = small.tile([1, S_LOC], fp16)
            for bi, (enc_t, s0) in enumerate(enc_tiles):
                blk = SBLKS[bi]
                nsl = blk // 512
                es = [
                    eps.tile([1, 512], fp32, tag="eps", name=f"eps{bi}{i}")
                    for i in range(nsl)
                ]
                for k in range(KCH):
                    for i in range(nsl):
                        nc.tensor.matmul(
                            es[i][:],
                            uT[:, k : k + 1],
                            enc_t[:, k, i * 512 : (i + 1) * 512],
                            start=(k == 0),
                            stop=(k == KCH - 1),
                        )
                for i in range(nsl):
                    dst = e_sb[:, s0 + i * 512 : s0 + (i + 1) * 512]
                    if i % 2 == 0:
                        nc.scalar.copy(dst, es[i][:])
                    else:
                        nc.vector.tensor_copy(dst, es[i][:])
                nc.scalar.dma_start(
                    e_part[s0 : s0 + blk].rearrange("(a c) -> a c", a=1),
                    e_sb[:, s0 : s0 + blk],
                )

            # ---- ship energies: ONE AllGather, fired post-stream ----
            nc.gpsimd.collective_compute(
                "AllGather",
                mybir.AluOpType.bypass,
                replica_groups=groups,
                ins=[e_part[:]],
                outs=[e_ag[:]],
            )

            # ---- combine column partials, then softmax (redundant) ----
            # e_ag = (r i): rank r = g*4+c holds partial e for s = g*8192+i.
            # Partition layout: p = g*64 + i//128, c'' = i%128 -> each
            # partition only needs its row-group's 4 ranks (half the
            # gather bytes, 256B descriptors), one load per ring in
            # parallel; and s = p*128 + c'', so the final store is a
            # plain contiguous [128,128].
            parts = small.tile([P, CG, S // P], fp16)
            for g in range(RG):
                src = e_ag[g * CG * S_LOC : (g + 1) * CG * S_LOC]
                src_v = src.rearrange("(r p c) -> p r c", r=CG, p=P // 2)
                ring = nc.sync if g == 0 else nc.scalar
                ring.dma_start(parts[g * (P // 2) : (g + 1) * (P // 2), :, :], src_v)
            qq = small.tile([P, 2, S // P], fp32)
            parts_v = parts[:].rearrange("p (x y) c -> p x y c", x=2)
            nc.vector.tensor_add(qq[:], parts_v[:, :, 0, :], parts_v[:, :, 1, :])
            ea = small.tile([P, S // P], fp32)
            qq_v = qq[:]
            nc.vector.tensor_add(ea[:], qq_v[:, 0, :], qq_v[:, 1, :])

            mx = small.tile([P, 1], fp32)
            nc.vector.reduce_max(mx[:], ea[:], axis=mybir.AxisListType.X)
            # global max: transpose [128,1] -> [1,128] on PE, reduce row 0
            mrow_ps = psA.tile([1, P], fp32, tag="mix", name="mrow_ps")
            nc.tensor.transpose(mrow_ps[:], mx[:], ident[:])
            gmax = small.tile([1, 1], fp32)
            nc.vector.reduce_max(gmax[:], mrow_ps[:], axis=mybir.AxisListType.X)
            # broadcast -gmax to [128,1] (negated ones fold the sign)
            gb_ps = psA.tile([P, 1], fp32, tag="mix", name="gb_ps")
            nc.tensor.matmul(gb_ps[:], neg_ones_row[:], gmax[:])
            nmx = small.tile([P, 1], fp32)
            nc.scalar.copy(nmx[:], gb_ps[:])
            # exp(e - gmax) with per-partition row sums in one ACT op
            xs = small.tile([P, S // P], fp32)
            sums = small.tile([P, 1], fp32)
            nc.scalar.activation(
                xs[:],
                ea[:],
                mybir.ActivationFunctionType.Exp,
                bias=nmx[:],
                scale=1.0,
                accum_out=sums[:],
            )
            # global sum: contract the partition axis on the PE
            tot_ps = psA.tile([1, 1], fp32, tag="mix", name="tot_ps")
            nc.tensor.matmul(tot_ps[:], ones_col[:], sums[:])
            rec = small.tile([1, 1], fp32)
            nc.vector.reciprocal(rec[:], tot_ps[:])
            rb_ps = psA.tile([P, 1], fp32, tag="mix", name="rb_ps")
            nc.tensor.matmul(rb_ps[:], ones_row[:], rec[:])
            outx = small.tile([P, S // P], fp16)
            nc.vector.tensor_scalar_mul(outx[:], xs[:], rb_ps[:])
            # s = p*128 + c'': contiguous 256B per partition; one half
            # per ring so the two stores drain in parallel
            out_v = attn_out.rearrange("(x p c) -> x p c", x=2, p=P // 2)
            outx_v = outx[:].rearrange("(x p) c -> x p c", x=2)
            nc.sync.dma_start(out_v[0], outx_v[0])
            nc.scalar.dma_start(out_v[1], outx_v[1])

    nc.compile()
    return nc


def _get_program():
    if "nc" not in _CACHE:
        _CACHE["nc"] = _build_program()
    return _CACHE["nc"]


def _make_in_maps(hidden, encoder_outputs, W):
    hidden = np.asarray(hidden, dtype=np.float32).astype(np.float16)
    enc = np.asarray(encoder_outputs, dtype=np.float32).astype(np.float16)
    W = np.asarray(W, dtype=np.float32).astype(np.float16)
    hid = hidden.reshape(NO, P).transpose(1, 0)  # hid[p, o] = hidden[o*128+p]
    # W as [p, o, h]: W_poh[p, o, h] = W[o*128+p, h]
    W_poh = W.reshape(NO, P, H).transpose(1, 0, 2)
    in_maps = []
    for r in range(NCORES):
        g, c = divmod(r, CG)
        wh = np.zeros((P, NO, WHW), dtype=np.float16)
        wh[:, :, 0:H_SH] = W_poh[:, :, c * H_SH : (c + 1) * H_SH]
        wh[:, :, H_SH] = hid
        shard = enc[g * S_LOC : (g + 1) * S_LOC, c * H_SH : (c + 1) * H_SH]
        # block-major slabs: encb[p, (blocks: k, s)] = enc[s0+s, k*128+p]
        encb = np.empty((P, S_LOC * KCH), dtype=np.float16)
        s0 = 0
        for blk in SBLKS:
            seg = shard[s0 : s0 + blk, :].T.reshape(KCH, P, blk).transpose(1, 0, 2)
            encb[:, s0 * KCH : (s0 + blk) * KCH] = seg.reshape(P, KCH * blk)
            s0 += blk
        in_maps.append({"encb": encb, "wh": wh})
    return in_maps


def run(hidden, encoder_outputs, W, b=None, trace=False):
    from concourse.bass_utils import run_bass_kernel_spmd

    nc = _get_program()
    in_maps = _make_in_maps(hidden, encoder_outputs, W)
    res = run_bass_kernel_spmd(nc, in_maps, list(range(NCORES)), trace=trace)
    out = np.asarray(res.results[0]["attn"], dtype=np.float32).reshape(1, 1, S)
    return out, res


def kernel(hidden, encoder_outputs, W, b):
    out, _ = run(hidden, encoder_outputs, W, b)
    return out
